# revision 1
# baseline (speedup 1.0000x reference)
"""Trainium2 Bass kernel for LIFNet (leaky-integrator net, no spiking).

Math: the module is linear, and the leaky integration L (a causal LTI filter
along T) commutes with the per-timestep linear layers:

    V2 = L(L(batch @ W1^T) @ W2^T) = (L^2)(batch @ (W2 @ W1)^T)

with Wc = W2 @ W1 of shape [10, 784].  L^2 has impulse response
h[m] = beta^2 (m-1) alpha^(m-2) (m >= 2), which decays below f32 noise by
lag ~128, so the filter is applied as a banded blocked matmul with two
constant 128x128 blocks (intra-block R0, previous-block R1).

Device work per core (13 b's of [2000, 784], data-parallel over batch):
  - z^T = Wc @ x^T via PE matmuls (Wc chunks stationary, K=112 d-chunks,
    N=500 t-columns), accumulated in PSUM.
  - z^T [10, 128] slices -> z [128, 10] via PE transpose (tiny, cheap).
  - V2^T[o, t'-block] = R1^T-term + R0^T-term via two K=128 PE matmuls.
  - V2^T [10, 2000] DMA'd out per b; host re-assembles [100, 2000, 10].

Input is host-pretransposed to [cores, 13, 112, 7, 2000] so DMA loads put the
contraction dim (d) on partitions with 2 KB contiguous runs at full HBM BW.
"""

import sys

import numpy as np

for _p in ("/opt/trn_rl_repo",):
    if _p not in sys.path:
        sys.path.append(_p)

B, T, DIN, H1, H2 = 100, 2000, 784, 100, 10
ALPHA, BETA = 0.7, 0.3

NCORES = 8
BPAD = 104          # batch padded to 8 * 13
BP = BPAD // NCORES  # 13 b's per core
DC = 112            # d-chunk width (784 = 7 * 112), partition dim of x tiles
NDC = DIN // DC     # 7
TG = 500            # t-columns per z-matmul group (N <= 512)
NTG = T // TG       # 4
TB = 128            # t'-block for the filter stage
NTB = (T + TB - 1) // TB  # 16
TPADF = NTB * TB    # 2048 free-dim padding for the z^T staging buffer

_CACHE: dict = {}


def _filter_blocks() -> np.ndarray:
    """R = [R1 | R0] as [128, 256] f32: rhs blocks for the filter matmuls.

    out[o, t'] += sum_tl z_block[tl, o] * R[tl, t'] with R[tl, t'] =
    h[lag], lag = (t' - tl) + 128 for R1 (z from previous t-block) and
    (t' - tl) for R0 (intra-block, strictly causal).
    """
    m = np.arange(512, dtype=np.float64)
    h = np.zeros(512)
    h[2:] = BETA * BETA * (m[2:] - 1.0) * ALPHA ** (m[2:] - 2.0)
    tl = np.arange(TB)[:, None]
    tp = np.arange(TB)[None, :]
    r1 = h[tp - tl + TB]
    lag0 = tp - tl
    r0 = np.where(lag0 >= 2, h[np.clip(lag0, 0, None)], 0.0)
    return np.concatenate([r1, r0], axis=1).astype(np.float32)


def _build(reps: int = 1):
    """Build + compile the per-core Bass kernel (shared by all 8 cores).

    reps>1 repeats the whole compute body (for benchmarking device time
    independent of the PJRT/axon dispatch floor)."""
    from contextlib import ExitStack

    import concourse.tile as tile
    from concourse import bacc, mybir

    f32 = mybir.dt.float32
    f32r = mybir.dt.float32r
    nc = bacc.Bacc(
        "TRN2", target_bir_lowering=False, debug=False, num_devices=NCORES
    )

    xT = nc.dram_tensor("xT", [BP, DC, NDC, T], f32r, kind="ExternalInput")
    wct = nc.dram_tensor("wct", [DC, NDC * H2], f32r, kind="ExternalInput")
    rh = nc.dram_tensor("rh", [TB, 2 * TB], f32, kind="ExternalInput")
    eye = nc.dram_tensor("eye", [H2, H2], f32, kind="ExternalInput")
    vout = nc.dram_tensor("vout", [BP, H2, T], f32, kind="ExternalOutput")

    with tile.TileContext(nc) as tc, ExitStack() as ctx:
        const = ctx.enter_context(tc.tile_pool(name="const", bufs=1))
        xpool = ctx.enter_context(tc.tile_pool(name="xp", bufs=3))
        ring = ctx.enter_context(tc.tile_pool(name="ring", bufs=1))
        zbp = ctx.enter_context(tc.tile_pool(name="zbp", bufs=2))
        vsb = ctx.enter_context(tc.tile_pool(name="vsb", bufs=2))
        zpsum = ctx.enter_context(tc.tile_pool(name="zps", bufs=2, space="PSUM"))
        tpsum = ctx.enter_context(tc.tile_pool(name="tps", bufs=3, space="PSUM"))
        vpsum = ctx.enter_context(tc.tile_pool(name="vps", bufs=3, space="PSUM"))

        wct_sb = const.tile([DC, NDC * H2], f32r, tag="wct")
        nc.sync.dma_start(wct_sb[:], wct.ap())
        rh_sb = const.tile([TB, 2 * TB], f32, tag="rh")
        nc.sync.dma_start(rh_sb[:], rh.ap())
        eye_sb = const.tile([H2, H2], f32, tag="eye")
        nc.sync.dma_start(eye_sb[:], eye.ap())

        # Two-deep manual ring: the t-pad cols (>=2000) of the z^T staging
        # tile must stay zero across b iterations, so memset only once.
        zts_ring = []
        for i in range(2):
            zt = ring.tile([H2, TPADF], f32, tag=f"zts{i}", name=f"zts{i}")
            nc.vector.memset(zt[:], 0.0)
            zts_ring.append(zt)

        for rep in range(reps):
          for b in range(BP):
            zts = zts_ring[b % 2]

            # z^T[o, t] = sum_d Wc[o, d] x[t, d], d-chunks of 112 on partitions
            # One 3.1 MB DMA per half-b (8 KB-run descriptors), two 500-col
            # matmul groups sliced from each half tile.
            for h in range(2):
                xt = xpool.tile([DC, NDC * (T // 2)], f32r, tag="xt")
                nc.sync.dma_start(
                    xt[:].rearrange("p (c t) -> p c t", c=NDC),
                    xT.ap()[b, :, :, h * (T // 2) : (h + 1) * (T // 2)],
                )
                for gg in range(2):
                    g = 2 * h + gg
                    zp = zpsum.tile([H2, TG], f32, tag="zp")
                    for c in range(NDC):
                        nc.tensor.matmul(
                            zp[:],
                            wct_sb[:, c * H2 : (c + 1) * H2],
                            xt[:, c * (T // 2) + gg * TG : c * (T // 2) + gg * TG + TG],
                            start=(c == 0),
                            stop=(c == NDC - 1),
                        )
                    nc.scalar.copy(zts[0:H2, g * TG : (g + 1) * TG], zp[:])

            # z[t, o] per 128-t-block via PE transpose of z^T slices
            zb = zbp.tile([TB, NTB * H2], f32, tag="zb")
            for j in range(NTB):
                ztp = tpsum.tile([TB, H2], f32, tag="ztp")
                nc.tensor.transpose(
                    ztp[:], zts[0:H2, j * TB : (j + 1) * TB], eye_sb[:]
                )
                nc.scalar.copy(zb[:, j * H2 : (j + 1) * H2], ztp[:])

            # V2^T[o, 128-t'-block] = sum over prev/current z t-blocks
            v2 = vsb.tile([H2, T], f32, tag="v2")
            for j in range(NTB):
                vp = vpsum.tile([H2, TB], f32, tag="vp")
                n_mm = 2 if j > 0 else 1
                mm = 0
                for roff, jj in ((0, j - 1), (TB, j)):
                    if jj < 0:
                        continue
                    nc.tensor.matmul(
                        vp[:],
                        zb[:, jj * H2 : (jj + 1) * H2],
                        rh_sb[:, roff : roff + TB],
                        start=(mm == 0),
                        stop=(mm == n_mm - 1),
                    )
                    mm += 1
                w = min(TB, T - j * TB)
                nc.scalar.copy(v2[0:H2, j * TB : j * TB + w], vp[0:H2, 0:w])

            nc.sync.dma_start(vout.ap()[b], v2[:])

    nc.compile()
    return nc


def _prep_inputs(batch: np.ndarray, W1: np.ndarray, W2: np.ndarray):
    wc = (W2.astype(np.float64) @ W1.astype(np.float64)).astype(np.float32)
    # [112, 7*10]: wct[p, c*10+o] = Wc[o, 112c + p]
    wct = np.ascontiguousarray(
        wc.T.reshape(NDC, DC, H2).transpose(1, 0, 2).reshape(DC, NDC * H2)
    )
    rh = _filter_blocks()
    eye = np.eye(H2, dtype=np.float32)

    bp = np.zeros((BPAD, T, DIN), np.float32)
    bp[:B] = batch
    # [8, 13, 112, 7, 2000]: core, b, d%112 (partitions), d-chunk, t
    xt = np.ascontiguousarray(
        bp.reshape(NCORES, BP, T, NDC, DC).transpose(0, 1, 4, 3, 2)
    )
    return xt, wct, rh, eye


def kernel(batch: np.ndarray, W1: np.ndarray, W2: np.ndarray) -> np.ndarray:
    from concourse import bass_utils

    if "nc" not in _CACHE:
        _CACHE["nc"] = _build()
    nc = _CACHE["nc"]

    xt, wct, rh, eye = _prep_inputs(batch, W1, W2)
    in_maps = [
        {"xT": xt[i], "wct": wct, "rh": rh, "eye": eye} for i in range(NCORES)
    ]
    res = bass_utils.run_bass_kernel_spmd(
        nc, in_maps, core_ids=list(range(NCORES)), **_CACHE.get("run_kwargs", {})
    )
    _CACHE["last_result"] = res

    full = np.concatenate([r["vout"] for r in res.results], axis=0)  # [104,10,2000]
    return np.ascontiguousarray(full.transpose(0, 2, 1)[:B])



# revision 3
# speedup vs baseline: 1.7082x; 1.7082x over previous
"""Trainium2 Bass kernel for LIFNet (leaky-integrator net, no spiking).

Math: the module is linear, and the leaky integration L (a causal LTI filter
along T) commutes with the per-timestep linear layers:

    V2 = L(L(batch @ W1^T) @ W2^T) = (L^2)(batch @ (W2 @ W1)^T)

with Wc = W2 @ W1 of shape [10, 784].  L^2 has impulse response
h[m] = beta^2 (m-1) alpha^(m-2) (m >= 2), which decays below f32 noise by
lag ~128, so the filter is applied as a banded blocked matmul with two
constant 128x128 blocks (intra-block R0, previous-block R1).

Device work per core (13 b's of [2000, 784], data-parallel over batch):
  - x is bf16 (host-converted) to halve HBM traffic; one DMA per b with
    per-partition-contiguous 28 KB runs so DMA is not descriptor-bound.
  - z^T = Wc @ x^T via PE matmuls, 7 d-chunks (K=112) issued into the 4
    column-groups of the PE array (tile_position=(0, 32q)) so up to 4
    chunk-matmuls stream concurrently; per-quadrant partials [106, 500]
    land in one PSUM tile and a tiny selector matmul (S[106, 10]) sums
    the quadrants into z^T [10, 500].
  - b's are processed in groups of 4, stacked at 32-partition offsets in
    the z^T staging tile [128, 2048], so the PE transpose ([128, 128])
    and the banded filter matmuls (M=128) amortize over 4 b's.
  - V2^T [10, 2000] slices DMA'd out per b on the scalar HWDGE queue;
    host re-assembles [100, 2000, 10].
"""

import sys

import numpy as np

for _p in ("/opt/trn_rl_repo",):
    if _p not in sys.path:
        sys.path.append(_p)

B, T, DIN, H1, H2 = 100, 2000, 784, 100, 10
ALPHA, BETA = 0.7, 0.3

NCORES = 8
BPAD = 104          # batch padded to 8 * 13
BP = BPAD // NCORES  # 13 b's per core
BGRP = 4            # b's stacked per stage-2/3 group (quadrant offsets)
DC = 112            # d-chunk width (784 = 7 * 112), partition dim of x tiles
NDC = DIN // DC     # 7
QS = 106            # stacked partials: quadrant q rows 32q .. 32q+9
TG = 500            # t-columns per z-matmul group (N <= 512)
NTG = T // TG       # 4
TB = 128            # t'-block for the filter stage
NTB = (T + TB - 1) // TB  # 16
TPADF = NTB * TB    # 2048 free-dim padding for the z^T staging buffer

_CACHE: dict = {}


def _filter_blocks() -> np.ndarray:
    """R = [R1 | R0] as [128, 256] f32: rhs blocks for the filter matmuls.

    out[o, t'] += sum_tl z_block[tl, o] * R[tl, t'] with R[tl, t'] =
    h[lag], lag = (t' - tl) + 128 for R1 (z from previous t-block) and
    (t' - tl) for R0 (intra-block, strictly causal).
    """
    m = np.arange(512, dtype=np.float64)
    h = np.zeros(512)
    h[2:] = BETA * BETA * (m[2:] - 1.0) * ALPHA ** (m[2:] - 2.0)
    tl = np.arange(TB)[:, None]
    tp = np.arange(TB)[None, :]
    r1 = h[tp - tl + TB]
    lag0 = tp - tl
    r0 = np.where(lag0 >= 2, h[np.clip(lag0, 0, None)], 0.0)
    return np.concatenate([r1, r0], axis=1).astype(np.float32)


def _build(reps: int = 1):
    """Build + compile the per-core Bass kernel (shared by all 8 cores)."""
    from contextlib import ExitStack

    import concourse.tile as tile
    from concourse import bacc, mybir

    f32 = mybir.dt.float32
    bf16 = mybir.dt.bfloat16
    nc = bacc.Bacc(
        "TRN2", target_bir_lowering=False, debug=False, num_devices=NCORES
    )

    xT = nc.dram_tensor("xT", [BP, DC, NDC, T], bf16, kind="ExternalInput")
    wct = nc.dram_tensor("wct", [DC, NDC * H2], bf16, kind="ExternalInput")
    rh = nc.dram_tensor("rh", [TB, 2 * TB], f32, kind="ExternalInput")
    eye = nc.dram_tensor("eye", [TB, TB], f32, kind="ExternalInput")
    sel = nc.dram_tensor("sel", [QS, H2], f32, kind="ExternalInput")
    vout = nc.dram_tensor("vout", [BP * H2, T], f32, kind="ExternalOutput")

    with tile.TileContext(nc) as tc, ExitStack() as ctx:
        const = ctx.enter_context(tc.tile_pool(name="const", bufs=1))
        xpool = ctx.enter_context(tc.tile_pool(name="xp", bufs=3))
        zsump = ctx.enter_context(tc.tile_pool(name="zsum", bufs=2))
        ring = ctx.enter_context(tc.tile_pool(name="ring", bufs=1))
        zbp = ctx.enter_context(tc.tile_pool(name="zbp", bufs=2))
        vsb = ctx.enter_context(tc.tile_pool(name="vsb", bufs=2))
        zps4 = ctx.enter_context(tc.tile_pool(name="zps4", bufs=2, space="PSUM"))
        zredp = ctx.enter_context(tc.tile_pool(name="zred", bufs=2, space="PSUM"))
        tpsum = ctx.enter_context(tc.tile_pool(name="tps", bufs=2, space="PSUM"))
        vpsum = ctx.enter_context(tc.tile_pool(name="vps", bufs=2, space="PSUM"))

        wct_sb = const.tile([DC, NDC * H2], bf16, tag="wct")
        nc.scalar.dma_start(wct_sb[:], wct.ap())
        rh_sb = const.tile([TB, 2 * TB], f32, tag="rh")
        nc.scalar.dma_start(rh_sb[:], rh.ap())
        eye_sb = const.tile([TB, TB], f32, tag="eye")
        nc.scalar.dma_start(eye_sb[:], eye.ap())
        sel_sb = const.tile([QS, H2], f32, tag="sel")
        nc.scalar.dma_start(sel_sb[:], sel.ap())

        # Two-deep manual ring: quadrant rows 32q+10..31 and t-pad cols
        # (>=2000) of the z^T staging tile must stay zero, memset once.
        zts_ring = []
        for i in range(2):
            zt = ring.tile([TB, TPADF], f32, tag=f"zts{i}", name=f"zts{i}")
            nc.vector.memset(zt[:], 0.0)
            zts_ring.append(zt)

        ngroups = (BP + BGRP - 1) // BGRP
        for rep in range(reps):
          for gi in range(ngroups):
            bs = list(range(gi * BGRP, min((gi + 1) * BGRP, BP)))
            zts = zts_ring[gi % 2]

            for q, b in enumerate(bs):
                xt = xpool.tile([DC, NDC * T], bf16, tag="xt")
                nc.sync.dma_start(
                    xt[:].rearrange("p (c t) -> p c t", c=NDC),
                    xT.ap()[b],
                )
                zsum = zsump.tile([QS, T], f32, tag="zsum")
                for g in range(NTG):
                    # 7 K=112 chunk-matmuls into 4 concurrent column
                    # groups; quadrant cq accumulates chunks cq and cq+4.
                    zp4 = zps4.tile([QS, TG], f32, tag="zp4")
                    for c in range(NDC):
                        cq = c % 4
                        nc.tensor.matmul(
                            zp4[32 * cq : 32 * cq + H2, :],
                            wct_sb[:, c * H2 : (c + 1) * H2],
                            xt[:, c * T + g * TG : c * T + g * TG + TG],
                            start=(c < 4),
                            stop=(c >= 3),
                            tile_position=(0, 32 * cq),
                        )
                    if g % 2 == 0:
                        nc.scalar.copy(
                            zsum[:, g * TG : (g + 1) * TG], zp4[:]
                        )
                    else:
                        nc.vector.tensor_copy(
                            zsum[:, g * TG : (g + 1) * TG], zp4[:]
                        )
                    # quadrant-sum: z^T[o, t] = sum_q zsum[32q+o, t]
                    zr = zredp.tile([H2, TG], f32, tag="zr")
                    nc.tensor.matmul(
                        zr[:],
                        sel_sb[:],
                        zsum[:, g * TG : (g + 1) * TG],
                        start=True,
                        stop=True,
                    )
                    if g % 2 == 0:
                        nc.vector.tensor_copy(
                            zts[32 * q : 32 * q + H2, g * TG : (g + 1) * TG],
                            zr[:],
                        )
                    else:
                        nc.scalar.copy(
                            zts[32 * q : 32 * q + H2, g * TG : (g + 1) * TG],
                            zr[:],
                        )

            # z[t, p] (p = 32q+o) per 128-t-block via PE transpose of the
            # group's stacked z^T rows.
            zb = zbp.tile([TB, NTB * TB], f32, tag="zb")
            for j in range(NTB):
                ztp = tpsum.tile([TB, TB], f32, tag="ztp")
                nc.tensor.transpose(
                    ztp[:], zts[:, j * TB : (j + 1) * TB], eye_sb[:]
                )
                if j % 2 == 0:
                    nc.scalar.copy(zb[:, j * TB : (j + 1) * TB], ztp[:])
                else:
                    nc.vector.tensor_copy(zb[:, j * TB : (j + 1) * TB], ztp[:])

            # V2^T[p, 128-t'-block] = sum over prev/current z t-blocks,
            # whole b-group at once (M = 128 output rows per matmul).
            v2 = vsb.tile([TB, T], f32, tag="v2")
            for j in range(NTB):
                vp = vpsum.tile([TB, TB], f32, tag="vp")
                n_mm = 2 if j > 0 else 1
                mm = 0
                for roff, jj in ((0, j - 1), (TB, j)):
                    if jj < 0:
                        continue
                    nc.tensor.matmul(
                        vp[:],
                        zb[:, jj * TB : (jj + 1) * TB],
                        rh_sb[:, roff : roff + TB],
                        start=(mm == 0),
                        stop=(mm == n_mm - 1),
                    )
                    mm += 1
                w = min(TB, T - j * TB)
                if j % 2 == 0:
                    nc.scalar.copy(v2[:, j * TB : j * TB + w], vp[:, 0:w])
                else:
                    nc.vector.tensor_copy(
                        v2[:, j * TB : j * TB + w], vp[:, 0:w]
                    )

            for q, b in enumerate(bs):
                nc.scalar.dma_start(
                    vout.ap()[b * H2 : (b + 1) * H2, :],
                    v2[32 * q : 32 * q + H2, :],
                )

    nc.compile()
    return nc


def _prep_inputs(batch: np.ndarray, W1: np.ndarray, W2: np.ndarray):
    import ml_dtypes

    bf16 = ml_dtypes.bfloat16
    wc = (W2.astype(np.float64) @ W1.astype(np.float64)).astype(np.float32)
    # [112, 7*10]: wct[p, c*10+o] = Wc[o, 112c + p]
    wct = np.ascontiguousarray(
        wc.T.reshape(NDC, DC, H2).transpose(1, 0, 2).reshape(DC, NDC * H2)
    ).astype(bf16)
    rh = _filter_blocks()
    eye = np.eye(TB, dtype=np.float32)
    sel = np.zeros((QS, H2), np.float32)
    for q in range(4):
        for i in range(H2):
            sel[32 * q + i, i] = 1.0

    bp = np.zeros((BPAD, T, DIN), np.float32)
    bp[:B] = batch
    # [8, 13, 112, 7, 2000]: core, b, d%112 (partitions), d-chunk, t
    xt = np.ascontiguousarray(
        bp.reshape(NCORES, BP, T, NDC, DC).transpose(0, 1, 4, 3, 2)
    ).astype(bf16)
    return xt, wct, rh, eye, sel


def kernel(batch: np.ndarray, W1: np.ndarray, W2: np.ndarray) -> np.ndarray:
    from concourse import bass_utils

    if "nc" not in _CACHE:
        _CACHE["nc"] = _build()
    nc = _CACHE["nc"]

    xt, wct, rh, eye, sel = _prep_inputs(batch, W1, W2)
    in_maps = [
        {"xT": xt[i], "wct": wct, "rh": rh, "eye": eye, "sel": sel}
        for i in range(NCORES)
    ]
    res = bass_utils.run_bass_kernel_spmd(
        nc, in_maps, core_ids=list(range(NCORES)), **_CACHE.get("run_kwargs", {})
    )
    _CACHE["last_result"] = res

    full = np.concatenate(
        [r["vout"].reshape(BP, H2, T) for r in res.results], axis=0
    )  # [104, 10, 2000]
    return np.ascontiguousarray(full.transpose(0, 2, 1)[:B])
